# revision 6
# baseline (speedup 1.0000x reference)
"""Trainium2 Bass kernel for nn_AutocorrelationCorrelogram.

For nervegram [B=4, F=50, T=20000, C=2]: 300 periodic-Hann-windowed frames
of length 512 per (b,f,c) signal, circular autocorrelation via
Wiener-Khinchin (rfft -> |.|^2 -> irfft), relu, normalize by sqrt(zero
lag), keep 256 lags, mean over channels -> [4, 50, 300, 256].

Sharding: pure data parallel over the 200 (b,f) pairs -> 25 per core x 8
cores (SPMD, no collectives).

Kernel structure (per core): 30 superbatches of 10 frames, channels merged
into 500-wide working tiles (cols = [c0 q0 | c0 q1 | c1 q0 | c1 q1], q =
5-frame group x 25 bf = 125 cols each):
  - DMA frames row-major (4KB contiguous rows), PE-transpose to
    time-major chunks trp_k [128 t, 500] (f32r transposes, 1.5 cyc/row)
  - radix-2 DIF split: g_e = w*f[0:256] + w*f[256:512], g_o = ... - ...
    The window+fold rides the PSUM->SBUF evacuation: v2 = w_hi (.) trp_hi
    (ACT copy with per-partition scale), then g_e/g_o via DVE
    scalar_tensor_tensor((trp_lo (.) w_lo) +/- v2). Halves the rfft
    matmul count (8 x 500-col matmuls instead of 16).
  - even bins = 256-DFT of g_e (E1 = Re, E2 = [Re X256 | Im]), odd bins
    = twiddled 256-DFT of g_o (O1 = Re, O2 = Im); squares on ACT
  - irfft flipped (D stationary, squares moving, acf^T [lags, cols] in
    PSUM): the P = Re^2+Im^2 add is folded into the PE accumulation by
    running each square through its own D matmul; bin-256 rides DE2
    row 0; channel-mean 0.5 folded into D scale (alpha=0.25)
  - relu -> bf16, PE transpose-back to [cols, lags], per-partition
    norm 1/sqrt(acf0), channel add via scalar_tensor_tensor, one
    output DMA per superbatch
"""

import sys

import numpy as np

sys.path.insert(0, "/opt/trn_rl_repo")

B, F, T, C = 4, 50, 20000, 2
NUM_FRAME = 300
LEN_FRAME = 512
LAGS = 256
N_CORES = 8
BF_PER_CORE = (B * F) // N_CORES  # 25

FRAMES_PER_SB = 10
N_SB_FULL = NUM_FRAME // FRAMES_PER_SB  # 30
NCOLS = 500  # 2c x 2q x 125

STARTS = np.linspace(0, T - LEN_FRAME, NUM_FRAME).astype(np.int64)


def build_weights():
    t = np.arange(256, dtype=np.float64)
    j = np.arange(128, dtype=np.float64)
    l = np.arange(LAGS, dtype=np.float64)
    tf = np.arange(LEN_FRAME, dtype=np.float64)
    w = 0.5 - 0.5 * np.cos(2.0 * np.pi * tf / LEN_FRAME)  # periodic hann

    # rfft of g_e (256-pt DFT, even bins 2j) / g_o (twiddled, odd bins 2j+1)
    we = np.zeros((256, 256))
    we[:, 0:128] = np.cos(2.0 * np.pi * np.outer(t, j) / 256.0)
    we[:, 128] = (-1.0) ** t  # Re X[256]
    we[:, 129:256] = -np.sin(2.0 * np.pi * np.outer(t, j[1:]) / 256.0)
    wo = np.zeros((256, 256))
    wo[:, 0:128] = np.cos(2.0 * np.pi * np.outer(t, 2 * j + 1) / 512.0)
    wo[:, 128:256] = -np.sin(2.0 * np.pi * np.outer(t, 2 * j + 1) / 512.0)

    # irfft (alpha folds the channel mean; output scales with sqrt(alpha))
    alpha = 0.25
    ang = 2.0 * np.pi * np.outer(2 * j, l) / 512.0
    de1 = (alpha / 512.0) * 2.0 * np.cos(ang)
    de1[0] *= 0.5  # bin 0 coef 1
    de2 = (alpha / 512.0) * 2.0 * np.cos(ang)
    de2[0] = (alpha / 512.0) * np.cos(np.pi * l)  # slot 0 carries bin 256
    dok = (alpha / 512.0) * 2.0 * np.cos(2.0 * np.pi * np.outer(2 * j + 1, l) / 512.0)

    f32 = np.float32
    return {
        "wea": we[0:128].astype(f32),
        "web": we[128:256].astype(f32),
        "woa": wo[0:128].astype(f32),
        "wob": wo[128:256].astype(f32),
        "de1": de1.astype(f32),
        "de2": de2.astype(f32),
        "dok": dok.astype(f32),
        "wv": np.ascontiguousarray(w.astype(f32).reshape(4, 128).T),  # [128,4]
        "eye": np.eye(128, dtype=f32),
    }


def build_nc(n_sb=N_SB_FULL):
    from contextlib import ExitStack

    import concourse.bacc as bacc
    import concourse.bass as bass
    import concourse.tile as tile
    from concourse import mybir

    f32 = mybir.dt.float32
    f32r = mybir.dt.float32r
    bf16 = mybir.dt.bfloat16
    AF = mybir.ActivationFunctionType
    ALU = mybir.AluOpType

    nc = bacc.Bacc("TRN2", target_bir_lowering=False, debug=False)

    x = nc.dram_tensor("x", [BF_PER_CORE, T, C], f32, kind="ExternalInput").ap()
    wea_d = nc.dram_tensor("wea", [128, 256], f32r, kind="ExternalInput").ap()
    web_d = nc.dram_tensor("web", [128, 256], f32r, kind="ExternalInput").ap()
    woa_d = nc.dram_tensor("woa", [128, 256], f32r, kind="ExternalInput").ap()
    wob_d = nc.dram_tensor("wob", [128, 256], f32r, kind="ExternalInput").ap()
    de1_d = nc.dram_tensor("de1", [128, 256], f32r, kind="ExternalInput").ap()
    de2_d = nc.dram_tensor("de2", [128, 256], f32r, kind="ExternalInput").ap()
    dok_d = nc.dram_tensor("dok", [128, 256], f32r, kind="ExternalInput").ap()
    wv_d = nc.dram_tensor("wv", [128, 4], f32, kind="ExternalInput").ap()
    eye_d = nc.dram_tensor("eye", [128, 128], f32, kind="ExternalInput").ap()
    eyeh_d = nc.dram_tensor("eyeh", [128, 128], bf16, kind="ExternalInput").ap()
    out = nc.dram_tensor(
        "out", [BF_PER_CORE, NUM_FRAME, LAGS], f32, kind="ExternalOutput"
    ).ap()

    with tile.TileContext(nc) as tc, ExitStack() as ctx:
        consts = ctx.enter_context(tc.tile_pool(name="consts", bufs=1))
        sb_pool = ctx.enter_context(tc.tile_pool(name="work", bufs=1))
        pp = ctx.enter_context(tc.tile_pool(name="ps", bufs=1, space="PSUM"))

        # ---- load constants once (eye first: gates the first transpose) ----
        eye_sb = consts.tile([128, 128], f32, tag="eye")
        nc.sync.dma_start(out=eye_sb[:], in_=eye_d[:])
        eyeh_sb = consts.tile([128, 128], bf16, tag="eyeh")
        nc.sync.dma_start(out=eyeh_sb[:], in_=eyeh_d[:])
        wv_sb = consts.tile([128, 4], f32, tag="wv")
        nc.sync.dma_start(out=wv_sb[:], in_=wv_d[:])
        wmat = {}
        for nm, d in [("wea", wea_d), ("web", web_d), ("woa", woa_d),
                      ("wob", wob_d), ("de1", de1_d), ("de2", de2_d),
                      ("dok", dok_d)]:
            t_ = consts.tile([128, 256], f32r, tag=nm)
            nc.sync.dma_start(out=t_[:], in_=d[:])
            wmat[nm] = t_
        zero_b = consts.tile([128, 1], f32, tag="zerob")
        nc.vector.memset(zero_b[:], 0.0)
        eps_b = consts.tile([128, 1], f32, tag="epsb")
        nc.vector.memset(eps_b[:], 1e-30)

        def load_sb(s):
            m0 = s * FRAMES_PER_SB
            tiles = []
            for q in range(2):
                ft = sb_pool.tile([125, LEN_FRAME, C], f32, tag="ft", bufs=8)
                mm = 0
                while mm < 5:
                    m = m0 + 5 * q + mm
                    run = 1
                    while (
                        mm + run < 5
                        and STARTS[m + run] - STARTS[m + run - 1]
                        == STARTS[m + 1] - STARTS[m]
                    ):
                        run += 1
                    s0 = int(STARTS[m])
                    step = int(STARTS[m + 1] - STARTS[m]) if run > 1 else 0
                    src_ap = bass.AP(
                        tensor=x.tensor,
                        offset=x.offset + s0 * C,
                        ap=[
                            [step * C, run],
                            [T * C, BF_PER_CORE],
                            [C, LEN_FRAME],
                            [1, C],
                        ],
                    )
                    nc.gpsimd.dma_start(
                        out=ft[25 * mm : 25 * (mm + run)], in_=src_ap
                    )
                    mm += run
                tiles.append(ft)
            return tiles

        PF = 3
        ft_queue = {}
        for s in range(min(PF, n_sb)):
            ft_queue[s] = load_sb(s)

        # cross-step state
        g_t = {}      # s -> dict of fold outputs (Ae, Ao, Be, Bo)
        sq_t = {}     # s -> [sqE1, sqE2, sqO1, sqO2]
        relu_t = {}   # s -> [reluT_h0, reluT_h1]

        WPAIR = [(0, 2, 0, 2), (1, 3, 1, 3)]  # (k_lo, k_hi, w_lo, w_hi)

        for i in range(n_sb + 2):
            s_f = i          # front: transpose + fold
            s_m = i - 1      # mid: rfft + squares + irfft + relu
            s_b = i - 2      # back: trback + norm + out

            trp = {}
            if s_f < n_sb:
                if s_f + PF < n_sb:
                    ft_queue[s_f + PF] = load_sb(s_f + PF)
                fts = ft_queue.pop(s_f)

                def do_tr(k):
                    tp = pp.tile([128, 2, 250], f32, tag="trp", bufs=3)
                    for c in range(C):
                        for q in range(2):
                            nc.tensor.transpose(
                                tp[:, c, 125 * q : 125 * q + 125],
                                fts[q][:, 128 * k : 128 * k + 128, c : c + 1],
                                eye_sb[:125, :125],
                            )
                    trp[k] = tp

                def do_fold(pair):
                    k_lo, k_hi, w_lo, w_hi = WPAIR[pair]
                    v2 = sb_pool.tile([128, NCOLS], f32, tag="v2", bufs=2)
                    nc.scalar.activation(
                        v2[:], trp[k_hi].rearrange("p c q -> p (c q)"),
                        AF.Copy, bias=0.0, scale=wv_sb[:, w_hi : w_hi + 1],
                    )
                    ge = sb_pool.tile([128, NCOLS], f32r, tag="g", bufs=8)
                    go = sb_pool.tile([128, NCOLS], f32r, tag="g", bufs=8)
                    lo = trp[k_lo].rearrange("p c q -> p (c q)")
                    nc.vector.scalar_tensor_tensor(
                        out=ge[:], in0=lo, scalar=wv_sb[:, w_lo : w_lo + 1],
                        in1=v2[:], op0=ALU.mult, op1=ALU.add,
                    )
                    nc.vector.scalar_tensor_tensor(
                        out=go[:], in0=lo, scalar=wv_sb[:, w_lo : w_lo + 1],
                        in1=v2[:], op0=ALU.mult, op1=ALU.subtract,
                    )
                    return ge, go

                do_tr(0)
                do_tr(2)
                gAe, gAo = do_fold(0)
                do_tr(1)

            if s_m >= 0 and s_m < n_sb:
                gm = g_t[s_m]
                sqs = []
                # E half: E1 (Re even), E2 ([ReX256 | Im even])
                for half, (wa, wb, gsel) in enumerate(
                    [("wea", "web", "e"), ("woa", "wob", "o")]
                ):
                    for colh in range(2):
                        fftp = pp.tile([128, NCOLS], f32, tag="fft", bufs=2)
                        sl = slice(128 * colh, 128 * colh + 128)
                        nc.tensor.matmul(
                            fftp[:], wmat[wa][:, sl], gm["A" + gsel][:],
                            start=True, stop=False,
                        )
                        nc.tensor.matmul(
                            fftp[:], wmat[wb][:, sl], gm["B" + gsel][:],
                            start=False, stop=True,
                        )
                        sq = sb_pool.tile([128, NCOLS], f32r, tag="sq", bufs=8)
                        nc.scalar.activation(
                            sq[:], fftp[:], AF.Square, bias=zero_b[:]
                        )
                        sqs.append(sq)
                    if half == 0 and s_f < n_sb:
                        # interleave front-sb tail into the PSUM gap
                        do_tr(3)
                        gBe, gBo = do_fold(1)
                        g_t[s_f] = {"Ae": gAe, "Ao": gAo, "Be": gBe, "Bo": gBo}
                sq_t[s_m] = sqs
                del g_t[s_m]
            elif s_f < n_sb:
                do_tr(3)
                gBe, gBo = do_fold(1)
                g_t[s_f] = {"Ae": gAe, "Ao": gAo, "Be": gBe, "Bo": gBo}

            if s_b >= 0:
                # trback: [lags, cols] bf16 -> [cols(125) x 4, 256 lags]
                rl = relu_t.pop(s_b)
                trb = pp.tile([125, 4, 256], bf16, tag="trb", bufs=1)
                for p in range(4):
                    for h in range(2):
                        nc.tensor.transpose(
                            trb[:, p, 128 * h : 128 * h + 128],
                            rl[h][:, 125 * p : 125 * p + 125],
                            eyeh_sb[:, :],
                        )
                rccs = []
                sqcs = []
                for p in range(4):
                    sqc = sb_pool.tile([125, 1], f32, tag="sqc", bufs=8)
                    nc.scalar.activation(
                        sqc[:], trb[:, p, 0:1], AF.Sqrt, bias=eps_b[:125]
                    )
                    sqcs.append(sqc)
                for p in range(4):
                    rcc = sb_pool.tile([125, 1], f32, tag="rcc", bufs=8)
                    nc.vector.reciprocal(out=rcc[:], in_=sqcs[p][:])
                    rccs.append(rcc)
                nts = []
                for g in range(2):
                    nt = sb_pool.tile([125, 256], f32, tag="nt", bufs=4)
                    nc.scalar.activation(
                        nt[:], trb[:, g, :], AF.Relu,
                        bias=zero_b[:125], scale=rccs[g][:],
                    )
                    nts.append(nt)
                m0 = s_b * FRAMES_PER_SB
                for g in range(2):
                    mt = sb_pool.tile([125, 256], f32, tag="mt", bufs=4)
                    nc.vector.scalar_tensor_tensor(
                        out=mt[:], in0=trb[:, 2 + g, :],
                        scalar=rccs[2 + g][:], in1=nts[g][:],
                        op0=ALU.mult, op1=ALU.add,
                    )
                    mf = m0 + 5 * g
                    nc.sync.dma_start(
                        out=out[:, mf : mf + 5, :].rearrange(
                            "bf mm l -> mm bf l"
                        ),
                        in_=mt[:],
                    )

            if s_m >= 0 and s_m < n_sb:
                # irfft flipped: acf^T [128 lags-half, 500]
                sqs = sq_t.pop(s_m)
                rls = []
                for h in range(2):
                    acfp = pp.tile([128, NCOLS], f32, tag="acfT", bufs=2)
                    sl = slice(128 * h, 128 * h + 128)
                    nc.tensor.matmul(
                        acfp[:], wmat["de1"][:, sl], sqs[0][:],
                        start=True, stop=False,
                    )
                    nc.tensor.matmul(
                        acfp[:], wmat["de2"][:, sl], sqs[1][:],
                        start=False, stop=False,
                    )
                    nc.tensor.matmul(
                        acfp[:], wmat["dok"][:, sl], sqs[2][:],
                        start=False, stop=False,
                    )
                    nc.tensor.matmul(
                        acfp[:], wmat["dok"][:, sl], sqs[3][:],
                        start=False, stop=True,
                    )
                    rlt = sb_pool.tile([128, NCOLS], bf16, tag="rl", bufs=4)
                    if h == 0:
                        nc.scalar.activation(
                            rlt[:], acfp[:], AF.Relu, bias=zero_b[:]
                        )
                    else:
                        nc.vector.tensor_scalar_max(rlt[:], acfp[:], 0.0)
                    rls.append(rlt)
                relu_t[s_m] = rls

    nc.compile()
    return nc


_NC_CACHE = {}


def _get_nc(n_sb=N_SB_FULL):
    if n_sb not in _NC_CACHE:
        _NC_CACHE[n_sb] = build_nc(n_sb)
    return _NC_CACHE[n_sb]


def make_in_maps(nerv):
    import ml_dtypes

    xs = nerv.reshape(B * F, T, C)
    wts = build_weights()
    base = {
        "wea": wts["wea"], "web": wts["web"],
        "woa": wts["woa"], "wob": wts["wob"],
        "de1": wts["de1"], "de2": wts["de2"], "dok": wts["dok"],
        "wv": wts["wv"], "eye": wts["eye"],
        "eyeh": wts["eye"].astype(ml_dtypes.bfloat16),
    }
    return [
        dict(
            base,
            x=np.ascontiguousarray(xs[BF_PER_CORE * i : BF_PER_CORE * (i + 1)]),
        )
        for i in range(N_CORES)
    ]


def kernel(nervegram, trace=False, **_ignored):
    from concourse.bass_utils import run_bass_kernel_spmd

    nerv = np.ascontiguousarray(np.asarray(nervegram, dtype=np.float32))
    assert nerv.shape == (B, F, T, C)
    in_maps = make_in_maps(nerv)
    nc = _get_nc()
    res = run_bass_kernel_spmd(nc, in_maps, list(range(N_CORES)), trace=trace)
    full = np.concatenate([res.results[i]["out"] for i in range(N_CORES)], axis=0)
    out = full.reshape(B, F, NUM_FRAME, LAGS)
    if trace:
        return out, res
    return out
